# revision 3
# baseline (speedup 1.0000x reference)
"""DifferentiableLogicLayer Trainium2 kernel.

Math: reference computes, per batch row t and gate g (G = INPUT_SIZE = 8192):
    a = x[t, g], b = x[t, (g+1) % 8192]            (x uniform in [0,1] -> clip no-op)
    out[t, g] = sum_o softmax(gate_logits[g])_o * op_o(a, b)
Each of the 16 soft ops is linear in {1, a, b, ab}, so with probs p:
    out = C0 + CA*a + CB*b + CAB*a*b
    C0  = p8+p9+p10+p11+p12+p13+p14+p15
    CA  = p2+p3+p6+p7-p12-p13
    CB  = p4+p5+p6+p7-p8-p9-p10-p11
    CAB = p1-p2-p4-2*p6-p7+p8+2*p9+p11+p13-p14
Factored as out = (CAB*a + CB)*b + (CA*a + C0): 4 VectorE ops + 2 GPSIMD ops
per element sweep.

Sharding: gates across the 8 cores (1024 gates each; gates are independent,
each needs x columns [g, g+1]).  Per core inputs:
    xs [2048, 1025] = x columns [1024c, 1024c+1024] (halo col, wraparound)
    gl [1024, 16]   = gate_logits rows for this core's gates
Coefficients are computed on-device (exp on ScalarE, subset reductions on
VectorE) and broadcast to all 128 partitions via K=1 matmuls into PSUM, where
VectorE reads them directly (keeps SBUF port rd1 free for GPSIMD).
"""

import numpy as np

NUM_GATES = 8192
INPUT_SIZE = 8192
BATCH = 2048
N_CORES = 8
G = NUM_GATES // N_CORES  # local gates per core: 1024
P = 128
NB = BATCH // P  # 16 batch tiles per core

_CACHE = {}


def _build_nc(reps=1):
    from contextlib import ExitStack

    import concourse.bacc as bacc
    import concourse.mybir as mybir
    from concourse.mybir import AluOpType as Op
    from concourse.tile import TileContext

    f32 = mybir.dt.float32
    Ax = mybir.AxisListType
    Act = mybir.ActivationFunctionType

    nc = bacc.Bacc("TRN2", target_bir_lowering=False, debug=False,
                   num_devices=N_CORES)
    xs = nc.dram_tensor("xs", [BATCH, G + 1], f32, kind="ExternalInput").ap()
    gl = nc.dram_tensor("gl", [G, 16], f32, kind="ExternalInput").ap()
    out = nc.dram_tensor("out", [BATCH, G], f32, kind="ExternalOutput").ap()

    with TileContext(nc) as tc, ExitStack() as ctx:
        cpool = ctx.enter_context(tc.tile_pool(name="coef", bufs=1))
        rpool = ctx.enter_context(tc.tile_pool(name="rows", bufs=1))
        ppool = ctx.enter_context(tc.tile_pool(name="psum", bufs=1, space="PSUM"))
        xpool = ctx.enter_context(tc.tile_pool(name="x", bufs=4))
        tpool = ctx.enter_context(tc.tile_pool(name="tmp", bufs=4))
        opool = ctx.enter_context(tc.tile_pool(name="o", bufs=4))

        for rep in range(reps):
            # ---- coefficient computation: [128, 8 gates x 16 ops] layout ----
            lg = cpool.tile([P, 8 * 16], f32, name=f"lg{rep}")
            nc.sync.dma_start(out=lg[:, :], in_=gl.rearrange("(p n) o -> p (n o)", p=P))
            E = cpool.tile([P, 8 * 16], f32, name=f"E{rep}")
            nc.scalar.activation(E[:, :], lg[:, :], Act.Exp)
            E3 = E[:, :].rearrange("p (n o) -> p n o", o=16)

            def red(sl, name):
                t = cpool.tile([P, 8], f32, name=name)
                nc.vector.tensor_reduce(t[:, :], sl, Ax.X, Op.add)
                return t

            den = red(E3[:, :, 0:16], f"den{rep}")
            n0 = red(E3[:, :, 8:16], f"n0{rep}")
            pa1 = red(E3[:, :, 2:4], f"pa1{rep}")
            pa2 = red(E3[:, :, 6:8], f"pa2{rep}")
            pa3 = red(E3[:, :, 8:10], f"pa3{rep}")
            pa4 = red(E3[:, :, 12:14], f"pa4{rep}")
            pb1 = red(E3[:, :, 4:8], f"pb1{rep}")
            pb2 = red(E3[:, :, 8:12], f"pb2{rep}")

            # CA = p2+p3+p6+p7-p8-p9-p12-p13
            na = cpool.tile([P, 8], f32, name=f"na{rep}")
            nc.vector.tensor_tensor(na[:, :], pa1[:, :], pa2[:, :], Op.add)
            nc.vector.tensor_tensor(na[:, :], na[:, :], pa3[:, :], Op.subtract)
            nc.vector.tensor_tensor(na[:, :], na[:, :], pa4[:, :], Op.subtract)
            nb = cpool.tile([P, 8], f32, name=f"nb{rep}")
            nc.vector.tensor_tensor(nb[:, :], pb1[:, :], pb2[:, :], Op.subtract)

            # NAB = E1 - E2 - E4 - 2*E6 - E7 + E8 + 2*E9 + E11 + E13 - E14
            def Eo(o):
                return E3[:, :, o]

            nab = cpool.tile([P, 8], f32, name=f"nab{rep}")
            # nab = (E6 * -2) + E1
            nc.vector.scalar_tensor_tensor(nab[:, :], Eo(6), -2.0, Eo(1), Op.mult, Op.add)
            t2 = cpool.tile([P, 8], f32, name=f"t2{rep}")
            # t2 = (E9 * 2) + E8
            nc.vector.scalar_tensor_tensor(t2[:, :], Eo(9), 2.0, Eo(8), Op.mult, Op.add)
            nc.vector.tensor_tensor(nab[:, :], nab[:, :], t2[:, :], Op.add)
            nc.vector.tensor_tensor(t2[:, :], Eo(11), Eo(13), Op.add)
            nc.vector.tensor_tensor(nab[:, :], nab[:, :], t2[:, :], Op.add)
            nc.vector.tensor_tensor(t2[:, :], Eo(2), Eo(4), Op.add)
            nc.vector.tensor_tensor(t2[:, :], t2[:, :], Eo(7), Op.add)
            nc.vector.tensor_tensor(t2[:, :], t2[:, :], Eo(14), Op.add)
            nc.vector.tensor_tensor(nab[:, :], nab[:, :], t2[:, :], Op.subtract)

            rden = cpool.tile([P, 8], f32, name=f"rden{rep}")
            nc.vector.reciprocal(rden[:, :], den[:, :])
            coefs = []
            for nm, t in (("c0", n0), ("ca", na), ("cb", nb), ("cab", nab)):
                c = cpool.tile([P, 8], f32, name=f"{nm}{rep}")
                nc.vector.tensor_tensor(c[:, :], t[:, :], rden[:, :], Op.mult)
                coefs.append(c)

            # ---- broadcast each coefficient to [128, G] in PSUM ----
            ones = rpool.tile([1, P], f32, name=f"ones{rep}")
            nc.vector.memset(ones[:, :], 1.0)
            R = {}
            for nm, c in zip(("c0", "ca", "cb", "cab"), coefs):
                row = rpool.tile([1, G], f32, name=f"row_{nm}{rep}")
                nc.sync.dma_start(out=row[:, :], in_=c[:, :])
                r = ppool.tile([P, G], f32, name=f"R_{nm}{rep}")
                for j in range(0, G, 512):
                    nc.tensor.matmul(r[:, j:j + 512], ones[:, :],
                                     row[:, j:j + 512], start=True, stop=True)
                R[nm] = r

            # ---- main loop: out = (CAB*a + CB)*b + (CA*a + C0) ----
            for bt in range(NB):
                xt = xpool.tile([P, G + 1], f32, name=f"xt{rep}_{bt}", tag="xt")
                nc.sync.dma_start(out=xt[:, :], in_=xs[bt * P:(bt + 1) * P, :])
                a = xt[:, 0:G]
                b = xt[:, 1:G + 1]
                u = tpool.tile([P, G], f32, name=f"u{rep}_{bt}", tag="u")
                nc.vector.tensor_tensor(u[:, :], a, R["cab"][:, :], Op.mult)
                nc.vector.tensor_tensor(u[:, :], u[:, :], R["cb"][:, :], Op.add)
                w = tpool.tile([P, G], f32, name=f"w{rep}_{bt}", tag="w")
                nc.gpsimd.tensor_tensor(w[:, :], u[:, :], b, Op.mult)
                v = tpool.tile([P, G], f32, name=f"v{rep}_{bt}", tag="v")
                nc.vector.tensor_tensor(v[:, :], a, R["ca"][:, :], Op.mult)
                nc.vector.tensor_tensor(v[:, :], v[:, :], R["c0"][:, :], Op.add)
                o = opool.tile([P, G], f32, name=f"o{rep}_{bt}", tag="o")
                nc.gpsimd.tensor_tensor(o[:, :], w[:, :], v[:, :], Op.add)
                nc.sync.dma_start(out=out[bt * P:(bt + 1) * P, :], in_=o[:, :])

    nc.compile()
    return nc


def _get_nc(reps=1):
    if reps not in _CACHE:
        _CACHE[reps] = _build_nc(reps)
    return _CACHE[reps]


def _shard_inputs(x, gate_logits):
    x = np.ascontiguousarray(x, dtype=np.float32)
    gate_logits = np.ascontiguousarray(gate_logits, dtype=np.float32)
    xs_full = np.concatenate([x, x[:, :1]], axis=1)  # wraparound halo
    in_maps = []
    for c in range(N_CORES):
        in_maps.append({
            "xs": np.ascontiguousarray(xs_full[:, c * G:c * G + G + 1]),
            "gl": np.ascontiguousarray(gate_logits[c * G:(c + 1) * G]),
        })
    return in_maps


def kernel(x, gate_logits):
    from concourse.bass_utils import run_bass_kernel_spmd

    nc = _get_nc()
    in_maps = _shard_inputs(x, gate_logits)
    res = run_bass_kernel_spmd(nc, in_maps, core_ids=list(range(N_CORES)))
    return np.concatenate([res.results[c]["out"] for c in range(N_CORES)], axis=1)
